# revision 2
# baseline (speedup 1.0000x reference)
"""Trainium2 Bass kernel for nn_DirectRegressionHead_46729244180888.

GRU encoder (2 layers, T=7 steps) + GRU decoder (12 steps) + projection heads,
B=32768 data-parallel over 8 NeuronCores (4096 batch per core).

Strategy:
  - Everything lives in [feature_on_partition, batch_on_free] layout, so the
    recurrence needs no transposes and biases are per-partition scalars.
  - Matmuls run in float32r (tf32-like, 1 cycle/row at N>=256); PSUM
    accumulates the x-part and h-part of each gate so gi+gh is free.
  - r/z gates: ACT sigmoid directly from PSUM with fused bias.
    n gate: tmp=(psum_hg+bhh_n)*r via one fused DVE scalar_tensor_tensor.
  - Step 0 is specialized (h=0): no h-matmuls, no zero-init needed.
  - ctx_proj (Cw, cb) is folded into the decoder input weights host-side,
    so ctx contributes one K=128 matmul per gate tile per step.
  - P2 and A2 are padded into one [.,6] output: rows = [delta(4), acorr(2)],
    written back as the next decoder carry with a single ACT op.
  - Per core the batch is processed in 2 passes of 2048 (SBUF fit), each
    pass in 4 chunks of 512 (PSUM bank sized) that pipeline across engines.
"""

import numpy as np

H = 256
CTX = 128
T = 8
TS = T - 1  # encoder steps
PRED = 12
NCORES = 8
B = 32768
BC = B // NCORES  # per-core batch
BP = 2048  # per-pass batch
NPASS = BC // BP
NB = 512  # chunk (PSUM bank) size
NCH = BP // NB
GD = 3 * H  # 768 gate rows

# name -> (shape, batch_sliced)
_INPUT_SPECS = {
    "ctxT": ([CTX, BC], True),
    "obs": ([TS, 6, BC], True),
    "dini": ([6, BC], True),
    "w0x": ([6, GD], False),
    "w0h": ([H, GD], False),
    "w1x": ([H, GD], False),
    "w1h": ([H, GD], False),
    "wsc": ([CTX, GD], False),
    "wsv": ([6, GD], False),
    "wsh": ([H, GD], False),
    "p1t": ([H, 128], False),
    "a1t": ([H, 64], False),
    "p2x": ([128, 6], False),
    "a2x": ([64, 6], False),
    "brz0": ([128, 4], False),
    "bxn0": ([128, 2], False),
    "bhn0": ([128, 2], False),
    "brz1": ([128, 4], False),
    "bxn1": ([128, 2], False),
    "bhn1": ([128, 2], False),
    "brzs": ([128, 4], False),
    "bxns": ([128, 2], False),
    "bhns": ([128, 2], False),
    "p1b": ([128, 1], False),
    "a1b": ([64, 1], False),
    "pab": ([6, 1], False),
}

_NC_CACHE = {}


def build_nc():
    if "nc" in _NC_CACHE:
        return _NC_CACHE["nc"]

    from contextlib import ExitStack

    import concourse.mybir as mybir
    import concourse.tile as tile
    from concourse import bacc

    dt = mybir.dt
    f32, f32r = dt.float32, dt.float32r
    AF = mybir.ActivationFunctionType
    OP = mybir.AluOpType

    nc = bacc.Bacc("TRN2", target_bir_lowering=False, debug=False, num_devices=NCORES)

    d = {
        name: nc.dram_tensor(name, shape, f32, kind="ExternalInput").ap()
        for name, (shape, _) in _INPUT_SPECS.items()
    }
    out_d = nc.dram_tensor("preds", [4 * PRED, BC], f32, kind="ExternalOutput").ap()

    with tile.TileContext(nc) as tc, ExitStack() as ctx:
        wpool = ctx.enter_context(tc.tile_pool(name="wpool", bufs=1))

        wr = {}  # float32r weight tiles
        bb = {}  # float32 bias tiles
        with tc.tile_pool(name="wstage", bufs=1) as wst:
            for name in [
                "w0x", "w0h", "w1x", "w1h", "wsc", "wsv", "wsh",
                "p1t", "a1t", "p2x", "a2x",
            ]:
                K, M = _INPUT_SPECS[name][0]
                if K > 128:
                    kt = K // 128
                    st = wst.tile([128, kt, M], f32, tag="wst_" + name)
                    nc.sync.dma_start(
                        out=st, in_=d[name].rearrange("(kt p) m -> p kt m", p=128)
                    )
                    wr[name] = wpool.tile([128, kt, M], f32r, tag=name, name="wr_" + name)
                else:
                    st = wst.tile([K, M], f32, tag="wst_" + name)
                    nc.sync.dma_start(out=st, in_=d[name])
                    wr[name] = wpool.tile([K, M], f32r, tag=name, name="wr_" + name)
                nc.vector.tensor_copy(out=wr[name], in_=st)

            for name in [
                "brz0", "bxn0", "bhn0", "brz1", "bxn1", "bhn1",
                "brzs", "bxns", "bhns", "p1b", "a1b", "pab",
            ]:
                bb[name] = wpool.tile(_INPUT_SPECS[name][0], f32, tag=name, name="bb_" + name)
                nc.sync.dma_start(out=bb[name], in_=d[name])

            ctx_st = wst.tile([CTX, BC], f32, tag="ctx_st")
            nc.sync.dma_start(out=ctx_st, in_=d["ctxT"])
            ctxr = wpool.tile([CTX, BC], f32r, tag="ctxr")
            nc.vector.tensor_copy(out=ctxr, in_=ctx_st)

        hpool = ctx.enter_context(tc.tile_pool(name="hpool", bufs=1))
        spool = ctx.enter_context(tc.tile_pool(name="spool", bufs=2))
        cpool = ctx.enter_context(tc.tile_pool(name="cpool", bufs=2))
        wk = ctx.enter_context(tc.tile_pool(name="wk", bufs=2))
        psp = ctx.enter_context(tc.tile_pool(name="psum", bufs=1, space="PSUM"))

        def gru_cell_chunk(c, xk_fn, wh, brz, bxn, bhn, h_tiles, step0):
            """Emit one batch-chunk of one GRU cell. h_tiles updated in place."""
            cs = slice(c * NB, (c + 1) * NB)
            # --- r, z gate tiles: full PSUM accumulation + fused-bias sigmoid
            r, z = [], []
            for g in range(4):
                ps = psp.tile([128, NB], f32, tag=f"rz{g}")
                mms = list(xk_fn(g, cs))
                if not step0:
                    gs = slice(g * 128, (g + 1) * 128)
                    mms += [(wh[:, kt, gs], h_tiles[kt][:, cs]) for kt in (0, 1)]
                for i, (lh, rh) in enumerate(mms):
                    nc.tensor.matmul(
                        ps, lh, rh, start=(i == 0), stop=(i == len(mms) - 1)
                    )
                out = wk.tile([128, NB], f32, tag=f"rz_o{g}")
                nc.scalar.activation(
                    out=out, in_=ps, func=AF.Sigmoid, bias=brz[:, g : g + 1]
                )
                (r if g < 2 else z).append(out)
            # --- n gate, x part -> ig (frees PSUM early, adds bias)
            ig = []
            for j in range(2):
                ps = psp.tile([128, NB], f32, tag=f"nx{j}")
                mms = list(xk_fn(4 + j, cs))
                for i, (lh, rh) in enumerate(mms):
                    nc.tensor.matmul(
                        ps, lh, rh, start=(i == 0), stop=(i == len(mms) - 1)
                    )
                igt = wk.tile([128, NB], f32, tag=f"ig{j}")
                nc.scalar.activation(
                    out=igt, in_=ps, func=AF.Identity, bias=bxn[:, j : j + 1]
                )
                ig.append(igt)
            # --- n gate, h part: tmp = (psum_hg + bhh_n) * r
            tmp = []
            for j in range(2):
                tm = wk.tile([128, NB], f32, tag=f"tmp{j}")
                if step0:
                    nc.vector.tensor_scalar_mul(
                        out=tm, in0=r[j], scalar1=bhn[:, j : j + 1]
                    )
                else:
                    gs = slice((4 + j) * 128, (5 + j) * 128)
                    ps = psp.tile([128, NB], f32, tag=f"nh{j}")
                    for kt in (0, 1):
                        nc.tensor.matmul(
                            ps,
                            wh[:, kt, gs],
                            h_tiles[kt][:, cs],
                            start=(kt == 0),
                            stop=(kt == 1),
                        )
                    nc.vector.scalar_tensor_tensor(
                        out=tm,
                        in0=ps,
                        scalar=bhn[:, j : j + 1],
                        in1=r[j],
                        op0=OP.add,
                        op1=OP.mult,
                    )
                tmp.append(tm)
            # --- n = tanh(ig + tmp); h' = n + z*(h-n)   (h=0 at step0)
            for j in range(2):
                st = wk.tile([128, NB], f32, tag=f"s{j}")
                nc.vector.tensor_add(out=st, in0=tmp[j], in1=ig[j])
                nt = wk.tile([128, NB], f32, tag=f"n{j}")
                nc.scalar.activation(out=nt, in_=st, func=AF.Tanh)
                if step0:
                    u = wk.tile([128, NB], f32, tag=f"d{j}")
                    nc.vector.tensor_mul(out=u, in0=z[j], in1=nt)
                    # h' = n - z*n, rounded to f32r by the fused STT
                    nc.vector.scalar_tensor_tensor(
                        out=h_tiles[j][:, cs],
                        in0=nt,
                        scalar=0.0,
                        in1=u,
                        op0=OP.add,
                        op1=OP.subtract,
                    )
                else:
                    dd = wk.tile([128, NB], f32, tag=f"d{j}")
                    nc.vector.tensor_sub(
                        out=dd, in0=h_tiles[j][:, cs].bitcast(f32), in1=nt
                    )
                    tt = wk.tile([128, NB], f32, tag=f"t{j}")
                    nc.vector.tensor_mul(out=tt, in0=z[j], in1=dd)
                    nc.vector.scalar_tensor_tensor(
                        out=h_tiles[j][:, cs],
                        in0=tt,
                        scalar=0.0,
                        in1=nt,
                        op0=OP.add,
                        op1=OP.add,
                    )

        for p in range(NPASS):
            pc = slice(p * BP, (p + 1) * BP)  # pass columns within BC

            h0 = [hpool.tile([128, BP], f32r, tag=f"h0{j}", name=f"h0{j}") for j in range(2)]
            h1 = [hpool.tile([128, BP], f32r, tag=f"h1{j}", name=f"h1{j}") for j in range(2)]

            # ---------------- encoder ----------------
            for t in range(TS):
                ost = spool.tile([6, BP], f32, tag="ost")
                nc.sync.dma_start(out=ost, in_=d["obs"][t, :, pc])
                osr = spool.tile([6, BP], f32r, tag="osr")
                nc.vector.tensor_copy(out=osr, in_=ost)

                def xk_l0(g, cs, _osr=osr):
                    return [
                        (wr["w0x"][:, g * 128 : (g + 1) * 128], _osr[:, cs])
                    ]

                def xk_l1(g, cs, _h0=h0):
                    gs = slice(g * 128, (g + 1) * 128)
                    return [
                        (wr["w1x"][:, kt, gs], _h0[kt][:, cs]) for kt in (0, 1)
                    ]

                for c in range(NCH):
                    gru_cell_chunk(
                        c, xk_l0, wr["w0h"], bb["brz0"], bb["bxn0"], bb["bhn0"],
                        h0, step0=(t == 0),
                    )
                for c in range(NCH):
                    gru_cell_chunk(
                        c, xk_l1, wr["w1h"], bb["brz1"], bb["bxn1"], bb["bhn1"],
                        h1, step0=(t == 0),
                    )

            # ---------------- decoder ----------------
            cst = spool.tile([6, BP], f32, tag="ost")
            nc.sync.dma_start(out=cst, in_=d["dini"][:, pc])
            cvam = cpool.tile([6, BP], f32r, tag="cvam")
            nc.vector.tensor_copy(out=cvam, in_=cst)

            for s in range(PRED):
                cvam_next = cpool.tile([6, BP], f32r, tag="cvam")

                def xk_dec(g, cs, _cv=cvam):
                    gs = slice(g * 128, (g + 1) * 128)
                    return [
                        (wr["wsc"][:, gs], ctxr[:, p * BP + cs.start : p * BP + cs.stop]),
                        (wr["wsv"][:, gs], _cv[:, cs]),
                    ]

                for c in range(NCH):
                    cs = slice(c * NB, (c + 1) * NB)
                    gru_cell_chunk(
                        c, xk_dec, wr["wsh"], bb["brzs"], bb["bxns"], bb["bhns"],
                        h1, step0=False,
                    )
                    # projections off the fresh h
                    ps1 = psp.tile([128, NB], f32, tag="nx0")
                    for kt in (0, 1):
                        nc.tensor.matmul(
                            ps1, wr["p1t"][:, kt, :], h1[kt][:, cs],
                            start=(kt == 0), stop=(kt == 1),
                        )
                    g1 = wk.tile([128, NB], f32r, tag="g1")
                    nc.scalar.activation(
                        out=g1, in_=ps1, func=AF.Gelu, bias=bb["p1b"][:, 0:1]
                    )
                    psa = psp.tile([64, NB], f32, tag="nx1")
                    for kt in (0, 1):
                        nc.tensor.matmul(
                            psa, wr["a1t"][:, kt, :], h1[kt][:, cs],
                            start=(kt == 0), stop=(kt == 1),
                        )
                    ga = wk.tile([64, NB], f32r, tag="ga")
                    nc.scalar.activation(
                        out=ga, in_=psa, func=AF.Gelu, bias=bb["a1b"][:, 0:1]
                    )
                    # combined [delta(4); acorr(2)] output
                    pso = psp.tile([6, NB], f32, tag="nh0")
                    nc.tensor.matmul(pso, wr["p2x"], g1, start=True, stop=False)
                    nc.tensor.matmul(pso, wr["a2x"], ga, start=False, stop=True)
                    pr = wk.tile([4, NB], f32, tag="pr")
                    nc.scalar.activation(
                        out=pr, in_=pso[0:4, :], func=AF.Identity,
                        bias=bb["pab"][0:4, :],
                    )
                    nc.sync.dma_start(
                        out=out_d[4 * s : 4 * s + 4, p * BP + cs.start : p * BP + cs.stop],
                        in_=pr,
                    )
                    nc.scalar.activation(
                        out=cvam_next[:, cs], in_=pso, func=AF.Identity,
                        bias=bb["pab"],
                    )
                cvam = cvam_next

    nc.compile()
    _NC_CACHE["nc"] = nc
    return nc


def prep_inputs(inputs):
    """Host-side prep: diffs, layout transposes, weight folds. Returns the
    full-[B] arrays keyed per _INPUT_SPECS."""
    f = np.float32
    ctx = np.asarray(inputs["ctx"], dtype=f)
    obs_traj = np.asarray(inputs["obs_traj"], dtype=f)
    obs_Me = np.asarray(inputs["obs_Me"], dtype=f)

    vel = obs_traj[1:] - obs_traj[:-1]  # [7, B, 2]
    me_vel = obs_Me[1:] - obs_Me[:-1]
    acc = vel[1:] - vel[:-1]  # [6, B, 2]
    acc = np.concatenate([acc[:1], acc], axis=0)  # [7, B, 2]
    obs_in = np.concatenate([vel, me_vel, acc], axis=-1)  # [7, B, 6]
    obs = np.ascontiguousarray(obs_in.transpose(0, 2, 1))  # [7, 6, B]

    last_vel = vel[-1]  # [B, 2]
    last_acc = vel[-1] - vel[-2]
    last_me = me_vel[-1]
    # carry rows: [cv(2), cm(2), ca(2)]
    dini = np.ascontiguousarray(
        np.concatenate([last_vel, last_me, last_acc], axis=-1).T
    )  # [6, B]

    ctxT = np.ascontiguousarray(ctx.T)  # [128, B]

    Wihs = np.asarray(inputs["Wihs"], dtype=f)
    Cw = np.asarray(inputs["Cw"], dtype=f)
    cb = np.asarray(inputs["cb"], dtype=f)
    wsc = np.ascontiguousarray((Wihs[:, :CTX] @ Cw).T)  # [128, 768]
    gctx = Wihs[:, :CTX] @ cb  # [768]
    # carry order [cv, cm, ca] maps to Wihs cols [128,129,132,133,130,131]
    wsv = np.ascontiguousarray(Wihs[:, [128, 129, 132, 133, 130, 131]].T)

    P2 = np.asarray(inputs["P2"], dtype=f)  # [4, 128]
    A2 = np.asarray(inputs["A2"], dtype=f)  # [2, 64]
    p2x = np.zeros((128, 6), dtype=f)
    p2x[:, :4] = P2.T
    a2x = np.zeros((64, 6), dtype=f)
    a2x[:, 4:6] = A2.T

    def col(b):  # [G] gate bias vector -> [128, G/128] tile
        v = np.asarray(b, dtype=f)
        return np.ascontiguousarray(v.reshape(-1, 128).T)

    bih0, bhh0 = inputs["bih0"], inputs["bhh0"]
    bih1, bhh1 = inputs["bih1"], inputs["bhh1"]
    bihs, bhhs = inputs["bihs"], inputs["bhhs"]

    arrs = {
        "ctxT": ctxT,
        "obs": obs,
        "dini": dini,
        "w0x": np.ascontiguousarray(np.asarray(inputs["Wih0"], dtype=f).T),
        "w0h": np.ascontiguousarray(np.asarray(inputs["Whh0"], dtype=f).T),
        "w1x": np.ascontiguousarray(np.asarray(inputs["Wih1"], dtype=f).T),
        "w1h": np.ascontiguousarray(np.asarray(inputs["Whh1"], dtype=f).T),
        "wsc": wsc,
        "wsv": wsv,
        "wsh": np.ascontiguousarray(np.asarray(inputs["Whhs"], dtype=f).T),
        "p1t": np.ascontiguousarray(np.asarray(inputs["P1"], dtype=f).T),
        "a1t": np.ascontiguousarray(np.asarray(inputs["A1"], dtype=f).T),
        "p2x": p2x,
        "a2x": a2x,
        "brz0": col(np.asarray(bih0)[:512] + np.asarray(bhh0)[:512]),
        "bxn0": col(np.asarray(bih0)[512:]),
        "bhn0": col(np.asarray(bhh0)[512:]),
        "brz1": col(np.asarray(bih1)[:512] + np.asarray(bhh1)[:512]),
        "bxn1": col(np.asarray(bih1)[512:]),
        "bhn1": col(np.asarray(bhh1)[512:]),
        "brzs": col(np.asarray(bihs)[:512] + np.asarray(bhhs)[:512] + gctx[:512]),
        "bxns": col(np.asarray(bihs)[512:] + gctx[512:]),
        "bhns": col(np.asarray(bhhs)[512:]),
        "p1b": np.ascontiguousarray(np.asarray(inputs["p1b"], dtype=f).reshape(128, 1)),
        "a1b": np.ascontiguousarray(np.asarray(inputs["a1b"], dtype=f).reshape(64, 1)),
        "pab": np.ascontiguousarray(
            np.concatenate([np.asarray(inputs["p2b"]), np.asarray(inputs["a2b"])]).astype(f).reshape(6, 1)
        ),
    }
    return arrs


def make_in_maps(arrs):
    in_maps = []
    for core in range(NCORES):
        cs = slice(core * BC, (core + 1) * BC)
        m = {}
        for name, (shape, sliced) in _INPUT_SPECS.items():
            a = arrs[name]
            m[name] = np.ascontiguousarray(a[..., cs]) if sliced else a
            assert list(m[name].shape) == shape, (name, m[name].shape, shape)
        in_maps.append(m)
    return in_maps


def assemble_output(results):
    preds = np.concatenate([r["preds"] for r in results], axis=1)  # [48, B]
    out = preds.reshape(PRED, 4, B).transpose(2, 0, 1)  # [B, 12, 4]
    return np.ascontiguousarray(out, dtype=np.float32)


def kernel(**inputs):
    from concourse.bass_utils import run_bass_kernel_spmd

    nc = build_nc()
    arrs = prep_inputs(inputs)
    in_maps = make_in_maps(arrs)
    res = run_bass_kernel_spmd(nc, in_maps, core_ids=list(range(NCORES)))
    return assemble_output(res.results)
